# revision 41
# baseline (speedup 1.0000x reference)
"""GQA attention (B=2, S=2048, D=2048, 32 q-heads / 8 kv-heads, hd=64),
tensor-parallel over the 8 kv-head groups on 8 NeuronCores.

Per-core math (core c owns kv head c and q heads 4c..4c+3):
  qT = (wq_c @ x.T), kT/vT likewise; RoPE via elementwise muls plus a
  constant pair-swap matmul R; scoresT[sk,sq] for both head-halves land
  in one 2-bank PSUM pair so a single exp covers both; ET = exp(s/8)
  with causal zeroing on diagonal tiles; out_pvT and the softmax
  denominator come from one matmul against [V | ones]; the output
  projection is weight-stationary (wo tile is lhsT) producing outT in
  [dim, token] layout, woven into the attention stream to fill the
  PE idle left by the scalar-bound exp; host sums partials and
  transposes once.

Schedule: all QKV/rope first (PE-dense, warms HAM, scalar idle), then
per (b, sqt) attention blocks with output-projection tiles of the
previous block interleaved between j-steps.
"""

from collections import deque
from contextlib import ExitStack

import ml_dtypes
import numpy as np

import concourse.bass as bass
import concourse.tile as tile
from concourse import bacc, mybir
from concourse import bass_utils
from concourse.bass_interp import get_hw_module

BF16 = mybir.dt.bfloat16
F32 = mybir.dt.float32

N_CORES = 8
B, S, DIM = 2, 2048, 2048
NH, NKV, HD = 32, 8, 64          # global heads
NHC = NH // N_CORES              # q heads per core = 4
QD = NHC * HD                    # per-core q out dim = 256
ST = B * S                       # total tokens = 4096
KT = DIM // 128                  # contraction k-tiles = 16
SQT = 512                        # sq tile (matmul free dim)
SKT = 128                        # sk tile (partition dim)
NSQ = S // SQT                   # sq tiles per batch = 4
NSK = S // SKT                   # sk tiles per batch = 16

_CACHE: dict = {}


def _build():
    if "nc" in _CACHE:
        return _CACHE["nc"]
    nc = bacc.Bacc(
        "TRN2",
        target_bir_lowering=False,
        debug=False,
        enable_asserts=False,
        num_devices=N_CORES,
    )
    xT = nc.dram_tensor("xt", [DIM, ST], BF16, kind="ExternalInput").ap()
    wqT = nc.dram_tensor("wqt", [DIM, QD], BF16, kind="ExternalInput").ap()
    wkvT = nc.dram_tensor("wkvt", [DIM, 2 * HD], BF16, kind="ExternalInput").ap()
    woT = nc.dram_tensor("wot", [QD, DIM], BF16, kind="ExternalInput").ap()
    cosE = nc.dram_tensor("cose", [128, ST], BF16, kind="ExternalInput").ap()
    sinE = nc.dram_tensor("sine", [128, ST], BF16, kind="ExternalInput").ap()
    r2t = nc.dram_tensor("r2t", [128, 128], BF16, kind="ExternalInput").ap()
    ident = nc.dram_tensor("ident", [64, 64], BF16, kind="ExternalInput").ap()
    out = nc.dram_tensor("out", [DIM, ST], BF16, kind="ExternalOutput").ap()

    with tile.TileContext(nc) as tc, ExitStack() as ctx:
        pers = ctx.enter_context(tc.tile_pool(name="pers", bufs=1))

        # -- persistent SBUF tensors ------------------------------------
        wq_ch = [pers.tile([128, 4 * QD], BF16, tag=f"wq{g}", name=f"wq{g}")
                 for g in range(4)]
        wkv_ch = [pers.tile([128, 4 * 2 * HD], BF16, tag=f"wkv{g}",
                            name=f"wkv{g}") for g in range(4)]
        wo_sb = [pers.tile([128, DIM], BF16, tag=f"wo{j}", name=f"wo{j}") for j in range(2)]
        cos_sb = pers.tile([128, ST], BF16, tag="cos")
        sin_sb = pers.tile([128, ST], BF16, tag="sin")
        r2t_sb = pers.tile([128, 128], BF16, tag="r2t")
        id_sb = pers.tile([64, 64], BF16, tag="ident")
        qrot = [pers.tile([128, ST], BF16, tag=f"qrot{t}", name=f"qrot{t}") for t in range(2)]
        krot = pers.tile([128, ST], BF16, tag="krot")  # k_rot duplicated in both halves
        vaug = pers.tile([128, B * NSK * 65], BF16, tag="vaug")
        attnT = [pers.tile([128, ST], BF16, tag=f"attnT{t}", name=f"attnT{t}") for t in range(2)]

        # startup DMA order matters: A(0) needs wq/wkv, r2t and the first
        # cos/sin chunk; wo and the later cos/sin chunks are only needed
        # much later, so they queue behind.
        wqT_v = wqT.rearrange("(t p) d -> p t d", p=128)
        wkvT_v = wkvT.rearrange("(t p) d -> p t d", p=128)
        for g in range(4):
            gs = slice(g * 4, (g + 1) * 4)
            nc.sync.dma_start(
                wq_ch[g].rearrange("p (t d) -> p t d", t=4), wqT_v[:, gs, :]
            )
            nc.sync.dma_start(
                wkv_ch[g].rearrange("p (t d) -> p t d", t=4), wkvT_v[:, gs, :]
            )
        nc.sync.dma_start(r2t_sb[:], r2t[:])
        nc.sync.dma_start(id_sb[:], ident[:])
        # cos/sin and wo go on the gpsimd DMA queue so weight staging
        # (sync) and x staging (scalar) are not stuck behind them
        for g in range(4):
            gs = bass.ts(g, ST // 4)
            nc.gpsimd.dma_start(cos_sb[:, gs], cosE[:, gs])
            nc.gpsimd.dma_start(sin_sb[:, gs], sinE[:, gs])
        for j in range(2):
            nc.gpsimd.dma_start(wo_sb[j][:], woT[j * 128:(j + 1) * 128, :])
        # ones column of V_aug (col 64 of each 65-wide block)
        nc.gpsimd.memset(vaug[:, 64::65], 1.0)

        with tc.tile_pool(name="xt", bufs=28) as xp, \
             tc.tile_pool(name="stage", bufs=3) as sp, \
             tc.tile_pool(name="et", bufs=6) as ep, \
             tc.tile_pool(name="misc", bufs=3) as mp, \
             tc.tile_pool(name="wout", bufs=6) as woutp, \
             tc.tile_pool(name="ps8", bufs=1, space="PSUM") as pool8:

            # preload the exp table set and the gpsimd ucode library (the
            # first partition_broadcast otherwise pays a ~7us IRAM load in
            # the middle of the attention stream) while startup DMAs run
            warm_in = sp.tile([1, 32], F32, tag="warm_in")
            nc.gpsimd.memset(warm_in[:], 0.0)
            warm_out = sp.tile([1, 32], BF16, tag="warm_out")
            nc.scalar.activation(
                warm_out[:], warm_in[:], mybir.ActivationFunctionType.Exp,
                scale=1.0,
            )
            warm_bc = sp.tile([64, 32], F32, tag="warm_bc")
            nc.gpsimd.partition_broadcast(warm_bc[:], warm_in[:])

            xt_cache = {}

            def emit_a(st):
                ss = bass.ts(st, SQT)
                psq2 = pool8.tile([128, 2 * SQT], F32, tag="big", name="psq2",
                                  bufs=2)
                pskv = pool8.tile([128, SQT], F32, tag="pskv", bufs=1)
                if st % 2 == 0:
                    xt_cache.clear()
                    for kt in range(KT):
                        t = xp.tile([128, 2 * SQT], BF16, name="xt_t")
                        nc.scalar.dma_start(
                            t[:], xT[kt * 128:(kt + 1) * 128,
                                     st * SQT:(st + 2) * SQT]
                        )
                        xt_cache[kt] = t
                for kt in range(KT):
                    xt_t = xt_cache[kt][:, (st % 2) * SQT:(st % 2 + 1) * SQT]
                    for dt in range(2):
                        nc.tensor.matmul(
                            psq2[:, dt * SQT:(dt + 1) * SQT],
                            wq_ch[kt // 4][:, (kt % 4) * QD + dt * 128:
                                           (kt % 4) * QD + (dt + 1) * 128],
                            xt_t[:],
                            start=(kt == 0),
                            stop=(kt == KT - 1),
                        )
                    nc.tensor.matmul(
                        pskv[:],
                        wkv_ch[kt // 4][:, (kt % 4) * 128:(kt % 4 + 1) * 128],
                        xt_t[:],
                        start=(kt == 0),
                        stop=(kt == KT - 1),
                    )
                # rope: the real/imag pairs interleave along the PARTITION
                # (feature) axis, so the pair-swap needs the PE (r2t matmul)
                qsb2 = sp.tile([128, 2 * SQT], BF16, tag="qsb2", name="qsb2")
                nc.vector.tensor_copy(qsb2[:], psq2[:])
                for dt in range(2):
                    qsb = qsb2[:, dt * SQT:(dt + 1) * SQT]
                    pr = pool8.tile([128, SQT], F32, tag="pr", name="pr", bufs=1)
                    nc.tensor.matmul(pr[:], r2t_sb[:], qsb)
                    t1 = sp.tile([128, SQT], BF16, tag="t1", name="t1")
                    nc.vector.tensor_mul(t1[:], qsb, cos_sb[:, ss])
                    t2 = sp.tile([128, SQT], BF16, tag="t2", name="t2")
                    nc.vector.tensor_mul(t2[:], pr[:], sin_sb[:, ss])
                    nc.vector.tensor_add(qrot[dt][:, ss], t1[:], t2[:])
                # rope on k (rows 0:64 of kv psum)
                ksb = sp.tile([64, SQT], BF16, tag="ksb", name="ksb")
                nc.vector.tensor_copy(ksb[:], pskv[0:64, :])
                prk_t = pool8.tile([128, SQT], F32, tag="pr", name="prk_t", bufs=1)
                prk = prk_t[0:64, :]
                nc.tensor.matmul(prk[:], r2t_sb[0:64, 0:64], ksb[:])
                t1k = sp.tile([64, SQT], BF16, tag="t1k", name="t1k")
                nc.vector.tensor_mul(t1k[:], ksb[:], cos_sb[0:64, ss])
                t2k = sp.tile([64, SQT], BF16, tag="t2k", name="t2k")
                nc.vector.tensor_mul(t2k[:], prk[:], sin_sb[0:64, ss])
                nc.vector.tensor_add(krot[0:64, ss], t1k[:], t2k[:])
                nc.vector.tensor_add(krot[64:128, ss], t1k[:], t2k[:])
                # V: transpose [64, 128] chunks -> vaug [128, 64] blocks
                vsb = sp.tile([64, SQT], BF16, tag="vsb", name="vsb")
                nc.vector.tensor_copy(vsb[:], pskv[64:128, :])
                for c in range(SQT // 128):
                    j = st * 4 + c  # global sk tile index
                    pt = pool8.tile([128, 64], BF16, tag="pr", name="pt", bufs=1)
                    nc.tensor.transpose(
                        pt[:], vsb[:, c * 128:(c + 1) * 128], id_sb[:]
                    )
                    nc.vector.tensor_copy(vaug[:, j * 65: j * 65 + 64], pt[:])

            # -- C phase: weight-stationary output projection ------------
            cq = deque()   # pending units: (b, sqt, ot)
            c_eng = [0]
            tail = [False]

            def emit_c_unit():
                b, sqt, ot = cq.popleft()
                t0 = b * S + sqt * SQT
                if tail[0]:
                    # A and B are done: the big tag's 2 ring slots join in
                    tag = ("pskv", "pr", "big", "big")[ot % 4]
                    bufs = 2 if tag == "big" else 1
                else:
                    tag = "pskv" if ot % 2 == 0 else "pr"
                    bufs = 1
                pw = pool8.tile([128, SQT], F32, tag=tag, name="pw", bufs=bufs)
                for jt in range(2):
                    nc.tensor.matmul(
                        pw[:],
                        wo_sb[jt][:, ot * 128:(ot + 1) * 128],
                        attnT[jt][:, t0:t0 + SQT],
                        start=(jt == 0),
                        stop=(jt == 1),
                    )
                osb = woutp.tile([128, SQT], BF16, tag="osb", name="osb")
                # scalar carries the serial exp chain during the attention
                # stream, so it only gets a third of the copies (half in
                # the drain tail where it idles)
                use_scalar = (c_eng[0] % 2 == 1) if tail[0] else (c_eng[0] % 3 == 2)
                if use_scalar:
                    nc.scalar.copy(osb[:], pw[:])
                else:
                    nc.vector.tensor_copy(osb[:], pw[:])
                c_eng[0] += 1
                nc.sync.dma_start(
                    out[ot * 128:(ot + 1) * 128, t0:t0 + SQT], osb[:]
                )

            def push_c(b, sqt):
                for ot in range(DIM // 128):
                    cq.append((b, sqt, ot))

            def emit_b(b, sqt, reserve=8, tail_block=False):
                n_sk = 4 * (sqt + 1)
                total_steps = 2 * n_sk
                navail = len(cq)
                paced = max(0, navail - reserve)
                step = [0]
                emitted = [0]

                def weave():
                    step[0] += 1
                    want = paced * step[0] // total_steps
                    while emitted[0] < want:
                        emit_c_unit()
                        emitted[0] += 1

                for dt in range(2):
                    sq0 = b * S + sqt * SQT
                    po2 = pool8.tile([65, 2 * SQT], F32, tag="po2",
                                     name="po2", bufs=1)
                    for j in range(n_sk):
                        sk0 = b * S + j * SKT
                        d = j - 4 * sqt
                        off = max(0, 128 * d)  # causally dead columns
                        w = SQT - off
                        ps2 = pool8.tile([128, 2 * SQT], F32, tag="big",
                                         name="ps2", bufs=2)
                        for hp in range(2):
                            hs = slice(hp * 64, (hp + 1) * 64)
                            nc.tensor.matmul(
                                ps2[:, hp * SQT + off:(hp + 1) * SQT],
                                krot[hs, sk0:sk0 + SKT],
                                qrot[dt][hs, sq0 + off:sq0 + SQT],
                                tile_position=(hp * 64, 0),
                            )
                        et2 = ep.tile([128, 2 * SQT], BF16, tag="et2",
                                      name="et2")
                        if off == 0:
                            nc.scalar.activation(
                                et2[:], ps2[:],
                                mybir.ActivationFunctionType.Exp,
                                scale=0.125,
                            )
                        else:
                            pv = ps2[:].rearrange(
                                "p (h w) -> p h w", h=2)[:, :, off:SQT]
                            ev = et2[:].rearrange(
                                "p (h w) -> p h w", h=2)[:, :, off:SQT]
                            nc.scalar.activation(
                                ev, pv,
                                mybir.ActivationFunctionType.Exp,
                                scale=0.125,
                            )
                        if d >= 0:  # diagonal tile: zero sk > sq
                            for hp in range(2):
                                nc.gpsimd.affine_select(
                                    out=et2[:, hp * SQT + off:(hp + 1) * SQT],
                                    in_=et2[:, hp * SQT + off:(hp + 1) * SQT],
                                    compare_op=mybir.AluOpType.is_ge,
                                    fill=0.0,
                                    base=0,
                                    channel_multiplier=-1,
                                    pattern=[[1, w]],
                                )
                        jj = b * NSK + j
                        for hp in range(2):
                            nc.tensor.matmul(
                                po2[:, hp * SQT + off:(hp + 1) * SQT],
                                vaug[:, jj * 65:(jj + 1) * 65],
                                et2[:, hp * SQT + off:(hp + 1) * SQT],
                                start=(j == 0),
                                stop=(j == n_sk - 1),
                            )
                        weave()
                    # burst C units so the PE has independent work queued
                    # while po2 drains and the next dt block's first PV
                    # waits on the po2 bank
                    for _ in range(reserve // 2):
                        if cq:
                            emit_c_unit()
                            emitted[0] += 1
                    # drain po2 (pv rows AND denominator row) in ONE scalar
                    # copy so the bank frees as fast as possible; the
                    # recip/broadcast/mul chain runs off SBUF afterwards
                    pocp = sp.tile([65, 2 * SQT], F32, tag="pocp", name="pocp",
                                   bufs=2)
                    nc.scalar.copy(pocp[:], po2[:])
                    den2 = mp.tile([1, 2 * SQT], F32, tag="den", name="den",
                                   bufs=1)
                    nc.vector.tensor_copy(den2[:], pocp[64:65, :])
                    recip2 = mp.tile([1, 2 * SQT], F32, tag="recip",
                                     name="recip", bufs=1)
                    nc.vector.reciprocal_approx_fast(recip2[:], den2[:])
                    bc2 = mp.tile([64, 2 * SQT], F32, tag="bc", name="bc",
                                  bufs=1)
                    nc.gpsimd.partition_broadcast(bc2[:], recip2[:])
                    for hp in range(2):
                        nc.vector.tensor_mul(
                            attnT[dt][hp * 64:(hp + 1) * 64, sq0:sq0 + SQT],
                            pocp[0:64, hp * SQT:(hp + 1) * SQT],
                            bc2[:, hp * SQT:(hp + 1) * SQT],
                        )

            # B(b,sqt) needs qkv through st = 4*b + sqt, so each B block
            # slots in right after its last prerequisite A block. B(0,0)
            # runs purely from SBUF and fills the x-DMA-starved ramp right
            # after A(0). Block granularity keeps the shared "big" PSUM
            # ring FIFO-consistent (no cross-block alloc inversions).
            emit_a(0)
            emit_b(0, 0)
            emit_a(1)
            emit_a(2)
            push_c(0, 0); emit_b(0, 1)
            emit_a(3)
            push_c(0, 1); emit_b(0, 2)
            emit_a(4)
            push_c(0, 2); emit_b(0, 3)
            emit_a(5)
            push_c(0, 3); emit_b(1, 0)
            emit_a(6)
            push_c(1, 0); emit_b(1, 1)
            emit_a(7)
            push_c(1, 1); emit_b(1, 2)
            push_c(1, 2); emit_b(1, 3, reserve=12, tail_block=True)
            push_c(1, 3)
            tail[0] = True
            while cq:
                emit_c_unit()

    nc.compile()
    nc.m = get_hw_module(nc.m)
    _CACHE["nc"] = nc
    return nc


def _prep_inputs(x, freqs_cos, freqs_sin, wq, wk, wv, wo):
    bf = ml_dtypes.bfloat16
    xT = np.ascontiguousarray(x.reshape(ST, DIM).T).astype(bf)
    # expanded rope tables in [feature, seq] layout, tiled over 2 head rows
    cos64 = np.repeat(freqs_cos.T, 2, axis=0)        # [64, S]
    sin64 = np.repeat(freqs_sin.T, 2, axis=0)
    cosE = np.tile(np.tile(cos64, (2, 1)), (1, B)).astype(bf)  # [128, ST]
    sinE = np.tile(np.tile(sin64, (2, 1)), (1, B)).astype(bf)
    # pair-swap matrix R (64x64), block-diagonal doubled, transposed
    R = np.zeros((64, 64), np.float32)
    for i in range(32):
        R[2 * i, 2 * i + 1] = -1.0
        R[2 * i + 1, 2 * i] = 1.0
    R2 = np.zeros((128, 128), np.float32)
    R2[:64, :64] = R
    R2[64:, 64:] = R
    r2t = np.ascontiguousarray(R2.T).astype(bf)
    ident = np.eye(64, dtype=np.float32).astype(bf)

    in_maps = []
    for c in range(N_CORES):
        wq_c = wq[c * QD:(c + 1) * QD, :]
        wk_c = wk[c * HD:(c + 1) * HD, :]
        wv_c = wv[c * HD:(c + 1) * HD, :]
        wkv_c = np.concatenate([wk_c, wv_c], axis=0)   # [128, DIM]
        wo_c = wo[:, c * QD:(c + 1) * QD]              # [DIM, 256]
        in_maps.append({
            "xt": xT,
            "wqt": np.ascontiguousarray(wq_c.T).astype(bf),
            "wkvt": np.ascontiguousarray(wkv_c.T).astype(bf),
            "wot": np.ascontiguousarray(wo_c.T).astype(bf),
            "cose": cosE,
            "sine": sinE,
            "r2t": r2t,
            "ident": ident,
        })
    return in_maps


def kernel(x, freqs_cos, freqs_sin, wq, wk, wv, wo, _trace=False, _trace_kwargs=None):
    x = np.asarray(x, dtype=np.float32)
    freqs_cos = np.asarray(freqs_cos, dtype=np.float32)
    freqs_sin = np.asarray(freqs_sin, dtype=np.float32)
    wq = np.asarray(wq, dtype=np.float32)
    wk = np.asarray(wk, dtype=np.float32)
    wv = np.asarray(wv, dtype=np.float32)
    wo = np.asarray(wo, dtype=np.float32)

    nc = _build()
    in_maps = _prep_inputs(x, freqs_cos, freqs_sin, wq, wk, wv, wo)
    kwargs = dict(_trace_kwargs or {})
    res = bass_utils.run_bass_kernel_spmd(
        nc, in_maps, core_ids=list(range(N_CORES)), trace=_trace, **kwargs
    )
    _CACHE["last_result"] = res
    acc = res.results[0]["out"].astype(np.float32)
    for c in range(1, N_CORES):
        acc += res.results[c]["out"].astype(np.float32)
    return np.ascontiguousarray(acc.T).reshape(B, S, DIM)


# revision 42
# speedup vs baseline: 1.0114x; 1.0114x over previous
"""GQA attention (B=2, S=2048, D=2048, 32 q-heads / 8 kv-heads, hd=64),
tensor-parallel over the 8 kv-head groups on 8 NeuronCores.

Per-core math (core c owns kv head c and q heads 4c..4c+3):
  qT = (wq_c @ x.T), kT/vT likewise; RoPE via elementwise muls plus a
  constant pair-swap matmul R; scoresT[sk,sq] for both head-halves land
  in one 2-bank PSUM pair so a single exp covers both; ET = exp(s/8)
  with causal zeroing on diagonal tiles; out_pvT and the softmax
  denominator come from one matmul against [V | ones]; the output
  projection is weight-stationary (wo tile is lhsT) producing outT in
  [dim, token] layout, woven into the attention stream to fill the
  PE idle left by the scalar-bound exp; host sums partials and
  transposes once.

Schedule: all QKV/rope first (PE-dense, warms HAM, scalar idle), then
per (b, sqt) attention blocks with output-projection tiles of the
previous block interleaved between j-steps.
"""

from collections import deque
from contextlib import ExitStack

import ml_dtypes
import numpy as np

import concourse.bass as bass
import concourse.tile as tile
from concourse import bacc, mybir
from concourse import bass_utils
from concourse.bass_interp import get_hw_module

BF16 = mybir.dt.bfloat16
F32 = mybir.dt.float32

N_CORES = 8
B, S, DIM = 2, 2048, 2048
NH, NKV, HD = 32, 8, 64          # global heads
NHC = NH // N_CORES              # q heads per core = 4
QD = NHC * HD                    # per-core q out dim = 256
ST = B * S                       # total tokens = 4096
KT = DIM // 128                  # contraction k-tiles = 16
SQT = 512                        # sq tile (matmul free dim)
SKT = 128                        # sk tile (partition dim)
NSQ = S // SQT                   # sq tiles per batch = 4
NSK = S // SKT                   # sk tiles per batch = 16

_CACHE: dict = {}


def _build():
    if "nc" in _CACHE:
        return _CACHE["nc"]
    nc = bacc.Bacc(
        "TRN2",
        target_bir_lowering=False,
        debug=False,
        enable_asserts=False,
        num_devices=N_CORES,
    )
    xT = nc.dram_tensor("xt", [DIM, ST], BF16, kind="ExternalInput").ap()
    wqT = nc.dram_tensor("wqt", [DIM, QD], BF16, kind="ExternalInput").ap()
    wkvT = nc.dram_tensor("wkvt", [DIM, 2 * HD], BF16, kind="ExternalInput").ap()
    woT = nc.dram_tensor("wot", [QD, DIM], BF16, kind="ExternalInput").ap()
    cosE = nc.dram_tensor("cose", [128, ST], BF16, kind="ExternalInput").ap()
    sinE = nc.dram_tensor("sine", [128, ST], BF16, kind="ExternalInput").ap()
    r2t = nc.dram_tensor("r2t", [128, 128], BF16, kind="ExternalInput").ap()
    ident = nc.dram_tensor("ident", [64, 64], BF16, kind="ExternalInput").ap()
    out = nc.dram_tensor("out", [DIM, ST], BF16, kind="ExternalOutput").ap()

    with tile.TileContext(nc) as tc, ExitStack() as ctx:
        pers = ctx.enter_context(tc.tile_pool(name="pers", bufs=1))

        # -- persistent SBUF tensors ------------------------------------
        wq_ch = [pers.tile([128, 4 * QD], BF16, tag=f"wq{g}", name=f"wq{g}")
                 for g in range(4)]
        wkv_ch = [pers.tile([128, 4 * 2 * HD], BF16, tag=f"wkv{g}",
                            name=f"wkv{g}") for g in range(4)]
        wo_sb = [pers.tile([128, DIM], BF16, tag=f"wo{j}", name=f"wo{j}") for j in range(2)]
        cos_sb = pers.tile([128, ST], BF16, tag="cos")
        sin_sb = pers.tile([128, ST], BF16, tag="sin")
        r2t_sb = pers.tile([128, 128], BF16, tag="r2t")
        id_sb = pers.tile([64, 64], BF16, tag="ident")
        qrot = [pers.tile([128, ST], BF16, tag=f"qrot{t}", name=f"qrot{t}") for t in range(2)]
        krot = pers.tile([128, ST], BF16, tag="krot")  # k_rot duplicated in both halves
        vaug = pers.tile([128, B * NSK * 65], BF16, tag="vaug")
        attnT = [pers.tile([128, ST], BF16, tag=f"attnT{t}", name=f"attnT{t}") for t in range(2)]

        # startup DMA order matters: A(0) needs wq/wkv, r2t and the first
        # cos/sin chunk; wo and the later cos/sin chunks are only needed
        # much later, so they queue behind.
        wqT_v = wqT.rearrange("(t p) d -> p t d", p=128)
        wkvT_v = wkvT.rearrange("(t p) d -> p t d", p=128)
        for g in range(4):
            gs = slice(g * 4, (g + 1) * 4)
            nc.sync.dma_start(
                wq_ch[g].rearrange("p (t d) -> p t d", t=4), wqT_v[:, gs, :]
            )
            nc.sync.dma_start(
                wkv_ch[g].rearrange("p (t d) -> p t d", t=4), wkvT_v[:, gs, :]
            )
        nc.sync.dma_start(r2t_sb[:], r2t[:])
        nc.sync.dma_start(id_sb[:], ident[:])
        # cos/sin and wo go on the gpsimd DMA queue so weight staging
        # (sync) and x staging (scalar) are not stuck behind them
        for g in range(4):
            gs = bass.ts(g, ST // 4)
            nc.gpsimd.dma_start(cos_sb[:, gs], cosE[:, gs])
            nc.gpsimd.dma_start(sin_sb[:, gs], sinE[:, gs])
        for j in range(2):
            nc.gpsimd.dma_start(wo_sb[j][:], woT[j * 128:(j + 1) * 128, :])
        # ones column of V_aug (col 64 of each 65-wide block)
        nc.gpsimd.memset(vaug[:, 64::65], 1.0)

        with tc.tile_pool(name="xt", bufs=28) as xp, \
             tc.tile_pool(name="stage", bufs=3) as sp, \
             tc.tile_pool(name="et", bufs=6) as ep, \
             tc.tile_pool(name="misc", bufs=3) as mp, \
             tc.tile_pool(name="wout", bufs=6) as woutp, \
             tc.tile_pool(name="ps8", bufs=1, space="PSUM") as pool8:

            # preload the exp table set and the gpsimd ucode library (the
            # first partition_broadcast otherwise pays a ~7us IRAM load in
            # the middle of the attention stream) while startup DMAs run
            warm_in = sp.tile([1, 32], F32, tag="warm_in")
            nc.gpsimd.memset(warm_in[:], 0.0)
            warm_out = sp.tile([1, 32], BF16, tag="warm_out")
            nc.scalar.activation(
                warm_out[:], warm_in[:], mybir.ActivationFunctionType.Exp,
                scale=1.0,
            )
            warm_bc = sp.tile([64, 32], F32, tag="warm_bc")
            nc.gpsimd.partition_broadcast(warm_bc[:], warm_in[:])

            xt_cache = {}

            def emit_a(st):
                ss = bass.ts(st, SQT)
                psq2 = pool8.tile([128, 2 * SQT], F32, tag="big", name="psq2",
                                  bufs=2)
                pskv = pool8.tile([128, SQT], F32, tag="pskv", bufs=1)
                if st % 2 == 0:
                    xt_cache.clear()
                    for kt in range(KT):
                        t = xp.tile([128, 2 * SQT], BF16, name="xt_t")
                        nc.scalar.dma_start(
                            t[:], xT[kt * 128:(kt + 1) * 128,
                                     st * SQT:(st + 2) * SQT]
                        )
                        xt_cache[kt] = t
                for kt in range(KT):
                    xt_t = xt_cache[kt][:, (st % 2) * SQT:(st % 2 + 1) * SQT]
                    for dt in range(2):
                        nc.tensor.matmul(
                            psq2[:, dt * SQT:(dt + 1) * SQT],
                            wq_ch[kt // 4][:, (kt % 4) * QD + dt * 128:
                                           (kt % 4) * QD + (dt + 1) * 128],
                            xt_t[:],
                            start=(kt == 0),
                            stop=(kt == KT - 1),
                        )
                    nc.tensor.matmul(
                        pskv[:],
                        wkv_ch[kt // 4][:, (kt % 4) * 128:(kt % 4 + 1) * 128],
                        xt_t[:],
                        start=(kt == 0),
                        stop=(kt == KT - 1),
                    )
                # rope: the real/imag pairs interleave along the PARTITION
                # (feature) axis, so the pair-swap needs the PE (r2t matmul)
                qsb2 = sp.tile([128, 2 * SQT], BF16, tag="qsb2", name="qsb2")
                nc.vector.tensor_copy(qsb2[:], psq2[:])
                for dt in range(2):
                    qsb = qsb2[:, dt * SQT:(dt + 1) * SQT]
                    pr = pool8.tile([128, SQT], F32, tag="pr", name="pr", bufs=1)
                    nc.tensor.matmul(pr[:], r2t_sb[:], qsb)
                    t1 = sp.tile([128, SQT], BF16, tag="t1", name="t1")
                    nc.vector.tensor_mul(t1[:], qsb, cos_sb[:, ss])
                    t2 = sp.tile([128, SQT], BF16, tag="t2", name="t2")
                    nc.vector.tensor_mul(t2[:], pr[:], sin_sb[:, ss])
                    nc.vector.tensor_add(qrot[dt][:, ss], t1[:], t2[:])
                # rope on k (rows 0:64 of kv psum)
                ksb = sp.tile([64, SQT], BF16, tag="ksb", name="ksb")
                nc.vector.tensor_copy(ksb[:], pskv[0:64, :])
                prk_t = pool8.tile([128, SQT], F32, tag="pr", name="prk_t", bufs=1)
                prk = prk_t[0:64, :]
                nc.tensor.matmul(prk[:], r2t_sb[0:64, 0:64], ksb[:])
                t1k = sp.tile([64, SQT], BF16, tag="t1k", name="t1k")
                nc.vector.tensor_mul(t1k[:], ksb[:], cos_sb[0:64, ss])
                t2k = sp.tile([64, SQT], BF16, tag="t2k", name="t2k")
                nc.vector.tensor_mul(t2k[:], prk[:], sin_sb[0:64, ss])
                nc.vector.tensor_add(krot[0:64, ss], t1k[:], t2k[:])
                nc.vector.tensor_add(krot[64:128, ss], t1k[:], t2k[:])
                # V: transpose [64, 128] chunks -> vaug [128, 64] blocks
                vsb = sp.tile([64, SQT], BF16, tag="vsb", name="vsb")
                nc.vector.tensor_copy(vsb[:], pskv[64:128, :])
                for c in range(SQT // 128):
                    j = st * 4 + c  # global sk tile index
                    pt = pool8.tile([128, 64], BF16, tag="pr", name="pt", bufs=1)
                    nc.tensor.transpose(
                        pt[:], vsb[:, c * 128:(c + 1) * 128], id_sb[:]
                    )
                    nc.vector.tensor_copy(vaug[:, j * 65: j * 65 + 64], pt[:])

            # -- C phase: weight-stationary output projection ------------
            cq = deque()   # pending units: (b, sqt, ot)
            c_eng = [0]
            tail = [False]

            def emit_c_unit():
                b, sqt, ot = cq.popleft()
                t0 = b * S + sqt * SQT
                if tail[0]:
                    # A and B are done: the big tag's 2 ring slots join in
                    tag = ("pskv", "pr", "big", "big")[ot % 4]
                    bufs = 2 if tag == "big" else 1
                else:
                    tag = "pskv" if ot % 2 == 0 else "pr"
                    bufs = 1
                pw = pool8.tile([128, SQT], F32, tag=tag, name="pw", bufs=bufs)
                for jt in range(2):
                    nc.tensor.matmul(
                        pw[:],
                        wo_sb[jt][:, ot * 128:(ot + 1) * 128],
                        attnT[jt][:, t0:t0 + SQT],
                        start=(jt == 0),
                        stop=(jt == 1),
                    )
                osb = woutp.tile([128, SQT], BF16, tag="osb", name="osb")
                # scalar carries the serial exp chain during the attention
                # stream, so it only gets a third of the copies (half in
                # the drain tail where it idles)
                use_scalar = (c_eng[0] % 2 == 1) if tail[0] else (c_eng[0] % 3 == 2)
                if use_scalar:
                    nc.scalar.copy(osb[:], pw[:])
                else:
                    nc.vector.tensor_copy(osb[:], pw[:])
                c_eng[0] += 1
                nc.sync.dma_start(
                    out[ot * 128:(ot + 1) * 128, t0:t0 + SQT], osb[:]
                )

            def push_c(b, sqt):
                for ot in range(DIM // 128):
                    cq.append((b, sqt, ot))

            def emit_b(b, sqt, reserve=8, tail_block=False):
                n_sk = 4 * (sqt + 1)
                total_steps = 2 * n_sk
                navail = len(cq)
                paced = max(0, navail - reserve)
                step = [0]
                emitted = [0]

                def weave():
                    step[0] += 1
                    want = paced * step[0] // total_steps
                    while emitted[0] < want:
                        emit_c_unit()
                        emitted[0] += 1

                for dt in range(2):
                    sq0 = b * S + sqt * SQT
                    po2 = pool8.tile([65, 2 * SQT], F32, tag="po2",
                                     name="po2", bufs=1)
                    for j in range(n_sk):
                        sk0 = b * S + j * SKT
                        d = j - 4 * sqt
                        off = max(0, 128 * d)  # causally dead columns
                        w = SQT - off
                        ps2 = pool8.tile([128, 2 * SQT], F32, tag="big",
                                         name="ps2", bufs=2)
                        for hp in range(2):
                            hs = slice(hp * 64, (hp + 1) * 64)
                            nc.tensor.matmul(
                                ps2[:, hp * SQT + off:(hp + 1) * SQT],
                                krot[hs, sk0:sk0 + SKT],
                                qrot[dt][hs, sq0 + off:sq0 + SQT],
                                tile_position=(hp * 64, 0),
                            )
                        et2 = ep.tile([128, 2 * SQT], BF16, tag="et2",
                                      name="et2")
                        if off == 0:
                            nc.scalar.activation(
                                et2[:], ps2[:],
                                mybir.ActivationFunctionType.Exp,
                                scale=0.125,
                            )
                        else:
                            pv = ps2[:].rearrange(
                                "p (h w) -> p h w", h=2)[:, :, off:SQT]
                            ev = et2[:].rearrange(
                                "p (h w) -> p h w", h=2)[:, :, off:SQT]
                            nc.scalar.activation(
                                ev, pv,
                                mybir.ActivationFunctionType.Exp,
                                scale=0.125,
                            )
                        if d >= 0:  # diagonal tile: zero sk > sq
                            for hp in range(2):
                                nc.gpsimd.affine_select(
                                    out=et2[:, hp * SQT + off:(hp + 1) * SQT],
                                    in_=et2[:, hp * SQT + off:(hp + 1) * SQT],
                                    compare_op=mybir.AluOpType.is_ge,
                                    fill=0.0,
                                    base=0,
                                    channel_multiplier=-1,
                                    pattern=[[1, w]],
                                )
                        jj = b * NSK + j
                        for hp in range(2):
                            nc.tensor.matmul(
                                po2[:, hp * SQT + off:(hp + 1) * SQT],
                                vaug[:, jj * 65:(jj + 1) * 65],
                                et2[:, hp * SQT + off:(hp + 1) * SQT],
                                start=(j == 0),
                                stop=(j == n_sk - 1),
                            )
                        weave()
                    # burst C units so the PE has independent work queued
                    # while po2 drains and the next dt block's first PV
                    # waits on the po2 bank
                    for _ in range(reserve // 2):
                        if cq:
                            emit_c_unit()
                            emitted[0] += 1
                    # drain po2 (pv rows AND denominator row) in ONE scalar
                    # copy so the bank frees as fast as possible; the
                    # recip/broadcast/mul chain runs off SBUF afterwards
                    pocp = sp.tile([65, 2 * SQT], F32, tag="pocp", name="pocp",
                                   bufs=2)
                    nc.scalar.copy(pocp[:], po2[:])
                    den2 = mp.tile([1, 2 * SQT], F32, tag="den", name="den",
                                   bufs=1)
                    nc.vector.tensor_copy(den2[:], pocp[64:65, :])
                    recip2 = mp.tile([1, 2 * SQT], F32, tag="recip",
                                     name="recip", bufs=1)
                    nc.vector.reciprocal_approx_fast(recip2[:], den2[:])
                    bc2 = mp.tile([64, 2 * SQT], F32, tag="bc", name="bc",
                                  bufs=1)
                    nc.gpsimd.partition_broadcast(bc2[:], recip2[:])
                    for hp in range(2):
                        nc.vector.tensor_mul(
                            attnT[dt][hp * 64:(hp + 1) * 64, sq0:sq0 + SQT],
                            pocp[0:64, hp * SQT:(hp + 1) * SQT],
                            bc2[:, hp * SQT:(hp + 1) * SQT],
                        )

            for st in range(4):
                emit_a(st)
            emit_b(0, 0)         # b=0 only needs st 0-3; starts the exp chain
            for st in range(4, 8):
                emit_a(st)
            push_c(0, 0); emit_b(0, 1)
            push_c(0, 1); emit_b(0, 2)
            push_c(0, 2); emit_b(0, 3)
            push_c(0, 3); emit_b(1, 0)
            push_c(1, 0); emit_b(1, 1)
            push_c(1, 1); emit_b(1, 2)
            push_c(1, 2); emit_b(1, 3, reserve=12, tail_block=True)
            push_c(1, 3)
            tail[0] = True
            while cq:
                emit_c_unit()

    nc.compile()
    nc.m = get_hw_module(nc.m)
    _CACHE["nc"] = nc
    return nc


def _prep_inputs(x, freqs_cos, freqs_sin, wq, wk, wv, wo):
    bf = ml_dtypes.bfloat16
    xT = np.ascontiguousarray(x.reshape(ST, DIM).T).astype(bf)
    # expanded rope tables in [feature, seq] layout, tiled over 2 head rows
    cos64 = np.repeat(freqs_cos.T, 2, axis=0)        # [64, S]
    sin64 = np.repeat(freqs_sin.T, 2, axis=0)
    cosE = np.tile(np.tile(cos64, (2, 1)), (1, B)).astype(bf)  # [128, ST]
    sinE = np.tile(np.tile(sin64, (2, 1)), (1, B)).astype(bf)
    # pair-swap matrix R (64x64), block-diagonal doubled, transposed
    R = np.zeros((64, 64), np.float32)
    for i in range(32):
        R[2 * i, 2 * i + 1] = -1.0
        R[2 * i + 1, 2 * i] = 1.0
    R2 = np.zeros((128, 128), np.float32)
    R2[:64, :64] = R
    R2[64:, 64:] = R
    r2t = np.ascontiguousarray(R2.T).astype(bf)
    ident = np.eye(64, dtype=np.float32).astype(bf)

    in_maps = []
    for c in range(N_CORES):
        wq_c = wq[c * QD:(c + 1) * QD, :]
        wk_c = wk[c * HD:(c + 1) * HD, :]
        wv_c = wv[c * HD:(c + 1) * HD, :]
        wkv_c = np.concatenate([wk_c, wv_c], axis=0)   # [128, DIM]
        wo_c = wo[:, c * QD:(c + 1) * QD]              # [DIM, 256]
        in_maps.append({
            "xt": xT,
            "wqt": np.ascontiguousarray(wq_c.T).astype(bf),
            "wkvt": np.ascontiguousarray(wkv_c.T).astype(bf),
            "wot": np.ascontiguousarray(wo_c.T).astype(bf),
            "cose": cosE,
            "sine": sinE,
            "r2t": r2t,
            "ident": ident,
        })
    return in_maps


def kernel(x, freqs_cos, freqs_sin, wq, wk, wv, wo, _trace=False, _trace_kwargs=None):
    x = np.asarray(x, dtype=np.float32)
    freqs_cos = np.asarray(freqs_cos, dtype=np.float32)
    freqs_sin = np.asarray(freqs_sin, dtype=np.float32)
    wq = np.asarray(wq, dtype=np.float32)
    wk = np.asarray(wk, dtype=np.float32)
    wv = np.asarray(wv, dtype=np.float32)
    wo = np.asarray(wo, dtype=np.float32)

    nc = _build()
    in_maps = _prep_inputs(x, freqs_cos, freqs_sin, wq, wk, wv, wo)
    kwargs = dict(_trace_kwargs or {})
    res = bass_utils.run_bass_kernel_spmd(
        nc, in_maps, core_ids=list(range(N_CORES)), trace=_trace, **kwargs
    )
    _CACHE["last_result"] = res
    acc = res.results[0]["out"].astype(np.float32)
    for c in range(1, N_CORES):
        acc += res.results[c]["out"].astype(np.float32)
    return np.ascontiguousarray(acc.T).reshape(B, S, DIM)


# revision 43
# speedup vs baseline: 1.0121x; 1.0007x over previous
"""GQA attention (B=2, S=2048, D=2048, 32 q-heads / 8 kv-heads, hd=64),
tensor-parallel over the 8 kv-head groups on 8 NeuronCores.

Per-core math (core c owns kv head c and q heads 4c..4c+3):
  qT = (wq_c @ x.T), kT/vT likewise; RoPE via elementwise muls plus a
  constant pair-swap matmul R; scoresT[sk,sq] for both head-halves land
  in one 2-bank PSUM pair so a single exp covers both; ET = exp(s/8)
  with causal zeroing on diagonal tiles; out_pvT and the softmax
  denominator come from one matmul against [V | ones]; the output
  projection is weight-stationary (wo tile is lhsT) producing outT in
  [dim, token] layout, woven into the attention stream to fill the
  PE idle left by the scalar-bound exp; host sums partials and
  transposes once.

Schedule: all QKV/rope first (PE-dense, warms HAM, scalar idle), then
per (b, sqt) attention blocks with output-projection tiles of the
previous block interleaved between j-steps.
"""

from collections import deque
from contextlib import ExitStack

import ml_dtypes
import numpy as np

import concourse.bass as bass
import concourse.tile as tile
from concourse import bacc, mybir
from concourse import bass_utils
from concourse.bass_interp import get_hw_module

BF16 = mybir.dt.bfloat16
F32 = mybir.dt.float32

N_CORES = 8
B, S, DIM = 2, 2048, 2048
NH, NKV, HD = 32, 8, 64          # global heads
NHC = NH // N_CORES              # q heads per core = 4
QD = NHC * HD                    # per-core q out dim = 256
ST = B * S                       # total tokens = 4096
KT = DIM // 128                  # contraction k-tiles = 16
SQT = 512                        # sq tile (matmul free dim)
SKT = 128                        # sk tile (partition dim)
NSQ = S // SQT                   # sq tiles per batch = 4
NSK = S // SKT                   # sk tiles per batch = 16

_CACHE: dict = {}


def _build():
    if "nc" in _CACHE:
        return _CACHE["nc"]
    nc = bacc.Bacc(
        "TRN2",
        target_bir_lowering=False,
        debug=False,
        enable_asserts=False,
        num_devices=N_CORES,
    )
    xT = nc.dram_tensor("xt", [DIM, ST], BF16, kind="ExternalInput").ap()
    wqT = nc.dram_tensor("wqt", [DIM, QD], BF16, kind="ExternalInput").ap()
    wkvT = nc.dram_tensor("wkvt", [DIM, 2 * HD], BF16, kind="ExternalInput").ap()
    woT = nc.dram_tensor("wot", [QD, DIM], BF16, kind="ExternalInput").ap()
    cosE = nc.dram_tensor("cose", [128, ST], BF16, kind="ExternalInput").ap()
    sinE = nc.dram_tensor("sine", [128, ST], BF16, kind="ExternalInput").ap()
    r2t = nc.dram_tensor("r2t", [128, 128], BF16, kind="ExternalInput").ap()
    ident = nc.dram_tensor("ident", [64, 64], BF16, kind="ExternalInput").ap()
    out = nc.dram_tensor("out", [DIM, ST], BF16, kind="ExternalOutput").ap()

    with tile.TileContext(nc) as tc, ExitStack() as ctx:
        pers = ctx.enter_context(tc.tile_pool(name="pers", bufs=1))

        # -- persistent SBUF tensors ------------------------------------
        wq_ch = [pers.tile([128, 4 * QD], BF16, tag=f"wq{g}", name=f"wq{g}")
                 for g in range(4)]
        wkv_ch = [pers.tile([128, 4 * 2 * HD], BF16, tag=f"wkv{g}",
                            name=f"wkv{g}") for g in range(4)]
        wo_sb = [pers.tile([128, DIM], BF16, tag=f"wo{j}", name=f"wo{j}") for j in range(2)]
        cos_sb = pers.tile([128, ST], BF16, tag="cos")
        sin_sb = pers.tile([128, ST], BF16, tag="sin")
        r2t_sb = pers.tile([128, 128], BF16, tag="r2t")
        id_sb = pers.tile([64, 64], BF16, tag="ident")
        qrot = [pers.tile([128, ST], BF16, tag=f"qrot{t}", name=f"qrot{t}") for t in range(2)]
        krot = pers.tile([128, ST], BF16, tag="krot")  # k_rot duplicated in both halves
        vaug = pers.tile([128, B * NSK * 65], BF16, tag="vaug")
        attnT = [pers.tile([128, ST], BF16, tag=f"attnT{t}", name=f"attnT{t}") for t in range(2)]

        # startup DMA order matters: A(0) needs wq/wkv, r2t and the first
        # cos/sin chunk; wo and the later cos/sin chunks are only needed
        # much later, so they queue behind.
        wqT_v = wqT.rearrange("(t p) d -> p t d", p=128)
        wkvT_v = wkvT.rearrange("(t p) d -> p t d", p=128)
        for g in range(4):
            gs = slice(g * 4, (g + 1) * 4)
            nc.sync.dma_start(
                wq_ch[g].rearrange("p (t d) -> p t d", t=4), wqT_v[:, gs, :]
            )
            nc.sync.dma_start(
                wkv_ch[g].rearrange("p (t d) -> p t d", t=4), wkvT_v[:, gs, :]
            )
        nc.sync.dma_start(r2t_sb[:], r2t[:])
        nc.sync.dma_start(id_sb[:], ident[:])
        # cos/sin and wo go on the gpsimd DMA queue so weight staging
        # (sync) and x staging (scalar) are not stuck behind them
        for g in range(4):
            gs = bass.ts(g, ST // 4)
            nc.gpsimd.dma_start(cos_sb[:, gs], cosE[:, gs])
            nc.gpsimd.dma_start(sin_sb[:, gs], sinE[:, gs])
        for j in range(2):
            nc.gpsimd.dma_start(wo_sb[j][:], woT[j * 128:(j + 1) * 128, :])
        # ones column of V_aug (col 64 of each 65-wide block)
        nc.gpsimd.memset(vaug[:, 64::65], 1.0)

        with tc.tile_pool(name="xt", bufs=28) as xp, \
             tc.tile_pool(name="stage", bufs=3) as sp, \
             tc.tile_pool(name="et", bufs=6) as ep, \
             tc.tile_pool(name="misc", bufs=3) as mp, \
             tc.tile_pool(name="wout", bufs=6) as woutp, \
             tc.tile_pool(name="ps8", bufs=1, space="PSUM") as pool8:

            # preload the exp table set and the gpsimd ucode library (the
            # first partition_broadcast otherwise pays a ~7us IRAM load in
            # the middle of the attention stream) while startup DMAs run
            warm_in = sp.tile([1, 32], F32, tag="warm_in")
            nc.gpsimd.memset(warm_in[:], 0.0)
            warm_out = sp.tile([1, 32], BF16, tag="warm_out")
            nc.scalar.activation(
                warm_out[:], warm_in[:], mybir.ActivationFunctionType.Exp,
                scale=1.0,
            )
            warm_bc = sp.tile([64, 32], F32, tag="warm_bc")
            nc.gpsimd.partition_broadcast(warm_bc[:], warm_in[:])

            xt_cache = {}

            def emit_a(st):
                ss = bass.ts(st, SQT)
                psq2 = pool8.tile([128, 2 * SQT], F32, tag="big", name="psq2",
                                  bufs=2)
                pskv = pool8.tile([128, SQT], F32, tag="pskv", bufs=1)
                if st % 2 == 0:
                    xt_cache.clear()
                    for kt in range(KT):
                        t = xp.tile([128, 2 * SQT], BF16, name="xt_t")
                        nc.scalar.dma_start(
                            t[:], xT[kt * 128:(kt + 1) * 128,
                                     st * SQT:(st + 2) * SQT]
                        )
                        xt_cache[kt] = t
                for kt in range(KT):
                    xt_t = xt_cache[kt][:, (st % 2) * SQT:(st % 2 + 1) * SQT]
                    for dt in range(2):
                        nc.tensor.matmul(
                            psq2[:, dt * SQT:(dt + 1) * SQT],
                            wq_ch[kt // 4][:, (kt % 4) * QD + dt * 128:
                                           (kt % 4) * QD + (dt + 1) * 128],
                            xt_t[:],
                            start=(kt == 0),
                            stop=(kt == KT - 1),
                        )
                    nc.tensor.matmul(
                        pskv[:],
                        wkv_ch[kt // 4][:, (kt % 4) * 128:(kt % 4 + 1) * 128],
                        xt_t[:],
                        start=(kt == 0),
                        stop=(kt == KT - 1),
                    )
                # rope: the real/imag pairs interleave along the PARTITION
                # (feature) axis, so the pair-swap needs the PE (r2t matmul)
                qsb2 = sp.tile([128, 2 * SQT], BF16, tag="qsb2", name="qsb2")
                nc.vector.tensor_copy(qsb2[:], psq2[:])
                for dt in range(2):
                    qsb = qsb2[:, dt * SQT:(dt + 1) * SQT]
                    pr = pool8.tile([128, SQT], F32, tag="pr", name="pr", bufs=1)
                    nc.tensor.matmul(pr[:], r2t_sb[:], qsb)
                    t1 = sp.tile([128, SQT], BF16, tag="t1", name="t1")
                    nc.vector.tensor_mul(t1[:], qsb, cos_sb[:, ss])
                    t2 = sp.tile([128, SQT], BF16, tag="t2", name="t2")
                    nc.vector.tensor_mul(t2[:], pr[:], sin_sb[:, ss])
                    nc.vector.tensor_add(qrot[dt][:, ss], t1[:], t2[:])
                # rope on k (rows 0:64 of kv psum)
                ksb = sp.tile([64, SQT], BF16, tag="ksb", name="ksb")
                nc.vector.tensor_copy(ksb[:], pskv[0:64, :])
                prk_t = pool8.tile([128, SQT], F32, tag="pr", name="prk_t", bufs=1)
                prk = prk_t[0:64, :]
                nc.tensor.matmul(prk[:], r2t_sb[0:64, 0:64], ksb[:])
                t1k = sp.tile([64, SQT], BF16, tag="t1k", name="t1k")
                nc.vector.tensor_mul(t1k[:], ksb[:], cos_sb[0:64, ss])
                t2k = sp.tile([64, SQT], BF16, tag="t2k", name="t2k")
                nc.vector.tensor_mul(t2k[:], prk[:], sin_sb[0:64, ss])
                nc.vector.tensor_add(krot[0:64, ss], t1k[:], t2k[:])
                nc.vector.tensor_add(krot[64:128, ss], t1k[:], t2k[:])
                # V: transpose [64, 128] chunks -> vaug [128, 64] blocks
                vsb = sp.tile([64, SQT], BF16, tag="vsb", name="vsb")
                nc.vector.tensor_copy(vsb[:], pskv[64:128, :])
                for c in range(SQT // 128):
                    j = st * 4 + c  # global sk tile index
                    pt = pool8.tile([128, 64], BF16, tag="pr", name="pt", bufs=1)
                    nc.tensor.transpose(
                        pt[:], vsb[:, c * 128:(c + 1) * 128], id_sb[:]
                    )
                    nc.vector.tensor_copy(vaug[:, j * 65: j * 65 + 64], pt[:])

            # -- C phase: weight-stationary output projection ------------
            cq = deque()   # pending units: (b, sqt, ot)
            c_eng = [0]
            tail = [False]

            def emit_c_unit():
                b, sqt, ot = cq.popleft()
                t0 = b * S + sqt * SQT
                if tail[0]:
                    # A and B are done: the big tag's 2 ring slots join in
                    tag = ("pskv", "pr", "big", "big")[ot % 4]
                    bufs = 2 if tag == "big" else 1
                else:
                    tag = "pskv" if ot % 2 == 0 else "pr"
                    bufs = 1
                pw = pool8.tile([128, SQT], F32, tag=tag, name="pw", bufs=bufs)
                for jt in range(2):
                    nc.tensor.matmul(
                        pw[:],
                        wo_sb[jt][:, ot * 128:(ot + 1) * 128],
                        attnT[jt][:, t0:t0 + SQT],
                        start=(jt == 0),
                        stop=(jt == 1),
                    )
                osb = woutp.tile([128, SQT], BF16, tag="osb", name="osb")
                # scalar carries the serial exp chain during the attention
                # stream, so it only gets a third of the copies (half in
                # the drain tail where it idles)
                use_scalar = (c_eng[0] % 2 == 1) if tail[0] else (c_eng[0] % 3 == 2)
                if use_scalar:
                    nc.scalar.copy(osb[:], pw[:])
                else:
                    nc.vector.tensor_copy(osb[:], pw[:])
                c_eng[0] += 1
                # in the drain tail, split the final DMAs across the sync
                # and (idle) gpsimd queues so they don't serialize
                dma_eng = nc.gpsimd if (tail[0] and ot % 2 == 1) else nc.sync
                dma_eng.dma_start(
                    out[ot * 128:(ot + 1) * 128, t0:t0 + SQT], osb[:]
                )

            def push_c(b, sqt):
                for ot in range(DIM // 128):
                    cq.append((b, sqt, ot))

            def emit_b(b, sqt, reserve=8, tail_block=False):
                n_sk = 4 * (sqt + 1)
                total_steps = 2 * n_sk
                navail = len(cq)
                paced = max(0, navail - reserve)
                step = [0]
                emitted = [0]

                def weave():
                    step[0] += 1
                    want = paced * step[0] // total_steps
                    while emitted[0] < want:
                        emit_c_unit()
                        emitted[0] += 1

                for dt in range(2):
                    sq0 = b * S + sqt * SQT
                    po2 = pool8.tile([65, 2 * SQT], F32, tag="po2",
                                     name="po2", bufs=1)
                    for j in range(n_sk):
                        sk0 = b * S + j * SKT
                        d = j - 4 * sqt
                        off = max(0, 128 * d)  # causally dead columns
                        w = SQT - off
                        ps2 = pool8.tile([128, 2 * SQT], F32, tag="big",
                                         name="ps2", bufs=2)
                        for hp in range(2):
                            hs = slice(hp * 64, (hp + 1) * 64)
                            nc.tensor.matmul(
                                ps2[:, hp * SQT + off:(hp + 1) * SQT],
                                krot[hs, sk0:sk0 + SKT],
                                qrot[dt][hs, sq0 + off:sq0 + SQT],
                                tile_position=(hp * 64, 0),
                            )
                        et2 = ep.tile([128, 2 * SQT], BF16, tag="et2",
                                      name="et2")
                        if off == 0:
                            nc.scalar.activation(
                                et2[:], ps2[:],
                                mybir.ActivationFunctionType.Exp,
                                scale=0.125,
                            )
                        else:
                            pv = ps2[:].rearrange(
                                "p (h w) -> p h w", h=2)[:, :, off:SQT]
                            ev = et2[:].rearrange(
                                "p (h w) -> p h w", h=2)[:, :, off:SQT]
                            nc.scalar.activation(
                                ev, pv,
                                mybir.ActivationFunctionType.Exp,
                                scale=0.125,
                            )
                        if d >= 0:  # diagonal tile: zero sk > sq
                            for hp in range(2):
                                nc.gpsimd.affine_select(
                                    out=et2[:, hp * SQT + off:(hp + 1) * SQT],
                                    in_=et2[:, hp * SQT + off:(hp + 1) * SQT],
                                    compare_op=mybir.AluOpType.is_ge,
                                    fill=0.0,
                                    base=0,
                                    channel_multiplier=-1,
                                    pattern=[[1, w]],
                                )
                        jj = b * NSK + j
                        for hp in range(2):
                            nc.tensor.matmul(
                                po2[:, hp * SQT + off:(hp + 1) * SQT],
                                vaug[:, jj * 65:(jj + 1) * 65],
                                et2[:, hp * SQT + off:(hp + 1) * SQT],
                                start=(j == 0),
                                stop=(j == n_sk - 1),
                            )
                        weave()
                    # burst C units so the PE has independent work queued
                    # while po2 drains and the next dt block's first PV
                    # waits on the po2 bank
                    for _ in range(reserve // 2):
                        if cq:
                            emit_c_unit()
                            emitted[0] += 1
                    # drain po2 (pv rows AND denominator row) in ONE scalar
                    # copy so the bank frees as fast as possible; the
                    # recip/broadcast/mul chain runs off SBUF afterwards
                    pocp = sp.tile([65, 2 * SQT], F32, tag="pocp", name="pocp",
                                   bufs=2)
                    nc.scalar.copy(pocp[:], po2[:])
                    den2 = mp.tile([1, 2 * SQT], F32, tag="den", name="den",
                                   bufs=1)
                    nc.vector.tensor_copy(den2[:], pocp[64:65, :])
                    recip2 = mp.tile([1, 2 * SQT], F32, tag="recip",
                                     name="recip", bufs=1)
                    nc.vector.reciprocal_approx_fast(recip2[:], den2[:])
                    bc2 = mp.tile([64, 2 * SQT], F32, tag="bc", name="bc",
                                  bufs=1)
                    nc.gpsimd.partition_broadcast(bc2[:], recip2[:])
                    for hp in range(2):
                        nc.vector.tensor_mul(
                            attnT[dt][hp * 64:(hp + 1) * 64, sq0:sq0 + SQT],
                            pocp[0:64, hp * SQT:(hp + 1) * SQT],
                            bc2[:, hp * SQT:(hp + 1) * SQT],
                        )

            for st in range(4):
                emit_a(st)
            emit_b(0, 0)         # b=0 only needs st 0-3; starts the exp chain
            for st in range(4, 8):
                emit_a(st)
            push_c(0, 0); emit_b(0, 1)
            push_c(0, 1); emit_b(0, 2)
            push_c(0, 2); emit_b(0, 3)
            push_c(0, 3); emit_b(1, 0)
            push_c(1, 0); emit_b(1, 1)
            push_c(1, 1); emit_b(1, 2)
            push_c(1, 2); emit_b(1, 3, reserve=12, tail_block=True)
            push_c(1, 3)
            tail[0] = True
            while cq:
                emit_c_unit()

    nc.compile()
    nc.m = get_hw_module(nc.m)
    _CACHE["nc"] = nc
    return nc


def _prep_inputs(x, freqs_cos, freqs_sin, wq, wk, wv, wo):
    bf = ml_dtypes.bfloat16
    xT = np.ascontiguousarray(x.reshape(ST, DIM).T).astype(bf)
    # expanded rope tables in [feature, seq] layout, tiled over 2 head rows
    cos64 = np.repeat(freqs_cos.T, 2, axis=0)        # [64, S]
    sin64 = np.repeat(freqs_sin.T, 2, axis=0)
    cosE = np.tile(np.tile(cos64, (2, 1)), (1, B)).astype(bf)  # [128, ST]
    sinE = np.tile(np.tile(sin64, (2, 1)), (1, B)).astype(bf)
    # pair-swap matrix R (64x64), block-diagonal doubled, transposed
    R = np.zeros((64, 64), np.float32)
    for i in range(32):
        R[2 * i, 2 * i + 1] = -1.0
        R[2 * i + 1, 2 * i] = 1.0
    R2 = np.zeros((128, 128), np.float32)
    R2[:64, :64] = R
    R2[64:, 64:] = R
    r2t = np.ascontiguousarray(R2.T).astype(bf)
    ident = np.eye(64, dtype=np.float32).astype(bf)

    in_maps = []
    for c in range(N_CORES):
        wq_c = wq[c * QD:(c + 1) * QD, :]
        wk_c = wk[c * HD:(c + 1) * HD, :]
        wv_c = wv[c * HD:(c + 1) * HD, :]
        wkv_c = np.concatenate([wk_c, wv_c], axis=0)   # [128, DIM]
        wo_c = wo[:, c * QD:(c + 1) * QD]              # [DIM, 256]
        in_maps.append({
            "xt": xT,
            "wqt": np.ascontiguousarray(wq_c.T).astype(bf),
            "wkvt": np.ascontiguousarray(wkv_c.T).astype(bf),
            "wot": np.ascontiguousarray(wo_c.T).astype(bf),
            "cose": cosE,
            "sine": sinE,
            "r2t": r2t,
            "ident": ident,
        })
    return in_maps


def kernel(x, freqs_cos, freqs_sin, wq, wk, wv, wo, _trace=False, _trace_kwargs=None):
    x = np.asarray(x, dtype=np.float32)
    freqs_cos = np.asarray(freqs_cos, dtype=np.float32)
    freqs_sin = np.asarray(freqs_sin, dtype=np.float32)
    wq = np.asarray(wq, dtype=np.float32)
    wk = np.asarray(wk, dtype=np.float32)
    wv = np.asarray(wv, dtype=np.float32)
    wo = np.asarray(wo, dtype=np.float32)

    nc = _build()
    in_maps = _prep_inputs(x, freqs_cos, freqs_sin, wq, wk, wv, wo)
    kwargs = dict(_trace_kwargs or {})
    res = bass_utils.run_bass_kernel_spmd(
        nc, in_maps, core_ids=list(range(N_CORES)), trace=_trace, **kwargs
    )
    _CACHE["last_result"] = res
    acc = res.results[0]["out"].astype(np.float32)
    for c in range(1, N_CORES):
        acc += res.results[c]["out"].astype(np.float32)
    return np.ascontiguousarray(acc.T).reshape(B, S, DIM)


# revision 44
# speedup vs baseline: 1.0267x; 1.0144x over previous
"""GQA attention (B=2, S=2048, D=2048, 32 q-heads / 8 kv-heads, hd=64),
tensor-parallel over the 8 kv-head groups on 8 NeuronCores.

Per-core math (core c owns kv head c and q heads 4c..4c+3):
  qT = (wq_c @ x.T), kT/vT likewise; RoPE via elementwise muls plus a
  constant pair-swap matmul R; scoresT[sk,sq] for both head-halves land
  in one 2-bank PSUM pair so a single exp covers both; ET = exp(s/8)
  with causal zeroing on diagonal tiles; out_pvT and the softmax
  denominator come from one matmul against [V | ones]; the output
  projection is weight-stationary (wo tile is lhsT) producing outT in
  [dim, token] layout, woven into the attention stream to fill the
  PE idle left by the scalar-bound exp; host sums partials and
  transposes once.

Schedule: all QKV/rope first (PE-dense, warms HAM, scalar idle), then
per (b, sqt) attention blocks with output-projection tiles of the
previous block interleaved between j-steps.
"""

from collections import deque
from contextlib import ExitStack

import ml_dtypes
import numpy as np

import concourse.bass as bass
import concourse.tile as tile
from concourse import bacc, mybir
from concourse import bass_utils
from concourse.bass_interp import get_hw_module

BF16 = mybir.dt.bfloat16
F32 = mybir.dt.float32

N_CORES = 8
B, S, DIM = 2, 2048, 2048
NH, NKV, HD = 32, 8, 64          # global heads
NHC = NH // N_CORES              # q heads per core = 4
QD = NHC * HD                    # per-core q out dim = 256
ST = B * S                       # total tokens = 4096
KT = DIM // 128                  # contraction k-tiles = 16
SQT = 512                        # sq tile (matmul free dim)
SKT = 128                        # sk tile (partition dim)
NSQ = S // SQT                   # sq tiles per batch = 4
NSK = S // SKT                   # sk tiles per batch = 16

_CACHE: dict = {}


def _build():
    if "nc" in _CACHE:
        return _CACHE["nc"]
    nc = bacc.Bacc(
        "TRN2",
        target_bir_lowering=False,
        debug=False,
        enable_asserts=False,
        num_devices=N_CORES,
    )
    xT = nc.dram_tensor("xt", [DIM, ST], BF16, kind="ExternalInput").ap()
    wqT = nc.dram_tensor("wqt", [DIM, QD], BF16, kind="ExternalInput").ap()
    wkvT = nc.dram_tensor("wkvt", [DIM, 2 * HD], BF16, kind="ExternalInput").ap()
    woT = nc.dram_tensor("wot", [QD, DIM], BF16, kind="ExternalInput").ap()
    cosE = nc.dram_tensor("cose", [128, ST], BF16, kind="ExternalInput").ap()
    sinE = nc.dram_tensor("sine", [128, ST], BF16, kind="ExternalInput").ap()
    r2t = nc.dram_tensor("r2t", [128, 128], BF16, kind="ExternalInput").ap()
    ident = nc.dram_tensor("ident", [64, 64], BF16, kind="ExternalInput").ap()
    out = nc.dram_tensor("out", [DIM, ST], BF16, kind="ExternalOutput").ap()

    with tile.TileContext(nc) as tc, ExitStack() as ctx:
        pers = ctx.enter_context(tc.tile_pool(name="pers", bufs=1))

        # -- persistent SBUF tensors ------------------------------------
        wq_ch = [pers.tile([128, 4 * QD], BF16, tag=f"wq{g}", name=f"wq{g}")
                 for g in range(4)]
        wkv_ch = [pers.tile([128, 4 * 2 * HD], BF16, tag=f"wkv{g}",
                            name=f"wkv{g}") for g in range(4)]
        wo_sb = [pers.tile([128, DIM], BF16, tag=f"wo{j}", name=f"wo{j}") for j in range(2)]
        cos_sb = pers.tile([128, ST], BF16, tag="cos")
        sin_sb = pers.tile([128, ST], BF16, tag="sin")
        r2t_sb = pers.tile([128, 128], BF16, tag="r2t")
        id_sb = pers.tile([64, 64], BF16, tag="ident")
        qrot = [pers.tile([128, ST], BF16, tag=f"qrot{t}", name=f"qrot{t}") for t in range(2)]
        krot = pers.tile([128, ST], BF16, tag="krot")  # k_rot duplicated in both halves
        vaug = pers.tile([128, B * NSK * 65], BF16, tag="vaug")
        attnT = [pers.tile([128, ST], BF16, tag=f"attnT{t}", name=f"attnT{t}") for t in range(2)]

        # startup DMA order matters: A(0) needs wq/wkv, r2t and the first
        # cos/sin chunk; wo and the later cos/sin chunks are only needed
        # much later, so they queue behind.
        wqT_v = wqT.rearrange("(t p) d -> p t d", p=128)
        wkvT_v = wkvT.rearrange("(t p) d -> p t d", p=128)
        for g in range(4):
            gs = slice(g * 4, (g + 1) * 4)
            nc.sync.dma_start(
                wq_ch[g].rearrange("p (t d) -> p t d", t=4), wqT_v[:, gs, :]
            )
            nc.sync.dma_start(
                wkv_ch[g].rearrange("p (t d) -> p t d", t=4), wkvT_v[:, gs, :]
            )
        nc.sync.dma_start(r2t_sb[:], r2t[:])
        nc.sync.dma_start(id_sb[:], ident[:])
        # cos/sin and wo go on the gpsimd DMA queue so weight staging
        # (sync) and x staging (scalar) are not stuck behind them
        for g in range(4):
            gs = bass.ts(g, ST // 4)
            nc.gpsimd.dma_start(cos_sb[:, gs], cosE[:, gs])
            nc.gpsimd.dma_start(sin_sb[:, gs], sinE[:, gs])
        for j in range(2):
            nc.gpsimd.dma_start(wo_sb[j][:], woT[j * 128:(j + 1) * 128, :])
        # ones column of V_aug (col 64 of each 65-wide block)
        nc.gpsimd.memset(vaug[:, 64::65], 1.0)

        with tc.tile_pool(name="xt", bufs=28) as xp, \
             tc.tile_pool(name="stage", bufs=3) as sp, \
             tc.tile_pool(name="et", bufs=6) as ep, \
             tc.tile_pool(name="misc", bufs=3) as mp, \
             tc.tile_pool(name="wout", bufs=6) as woutp, \
             tc.tile_pool(name="ps8", bufs=1, space="PSUM") as pool8:

            # preload the exp table set and the gpsimd ucode library (the
            # first partition_broadcast otherwise pays a ~7us IRAM load in
            # the middle of the attention stream) while startup DMAs run
            warm_in = sp.tile([1, 32], F32, tag="warm_in")
            nc.gpsimd.memset(warm_in[:], 0.0)
            warm_out = sp.tile([1, 32], BF16, tag="warm_out")
            nc.scalar.activation(
                warm_out[:], warm_in[:], mybir.ActivationFunctionType.Exp,
                scale=1.0,
            )
            warm_bc = sp.tile([64, 32], F32, tag="warm_bc")
            nc.gpsimd.partition_broadcast(warm_bc[:], warm_in[:])

            xt_cache = {}

            def emit_a(st):
                ss = bass.ts(st, SQT)
                psq2 = pool8.tile([128, 2 * SQT], F32, tag="big", name="psq2",
                                  bufs=2)
                pskv = pool8.tile([128, SQT], F32, tag="pskv", bufs=1)
                if st % 2 == 0:
                    xt_cache.clear()
                    for kt in range(KT):
                        t = xp.tile([128, 2 * SQT], BF16, name="xt_t")
                        nc.scalar.dma_start(
                            t[:], xT[kt * 128:(kt + 1) * 128,
                                     st * SQT:(st + 2) * SQT]
                        )
                        xt_cache[kt] = t
                for kt in range(KT):
                    xt_t = xt_cache[kt][:, (st % 2) * SQT:(st % 2 + 1) * SQT]
                    for dt in range(2):
                        nc.tensor.matmul(
                            psq2[:, dt * SQT:(dt + 1) * SQT],
                            wq_ch[kt // 4][:, (kt % 4) * QD + dt * 128:
                                           (kt % 4) * QD + (dt + 1) * 128],
                            xt_t[:],
                            start=(kt == 0),
                            stop=(kt == KT - 1),
                        )
                    nc.tensor.matmul(
                        pskv[:],
                        wkv_ch[kt // 4][:, (kt % 4) * 128:(kt % 4 + 1) * 128],
                        xt_t[:],
                        start=(kt == 0),
                        stop=(kt == KT - 1),
                    )
                # rope: the real/imag pairs interleave along the PARTITION
                # (feature) axis, so the pair-swap needs the PE (r2t matmul)
                qsb2 = sp.tile([128, 2 * SQT], BF16, tag="qsb2", name="qsb2")
                nc.vector.tensor_copy(qsb2[:], psq2[:])
                for dt in range(2):
                    qsb = qsb2[:, dt * SQT:(dt + 1) * SQT]
                    pr = pool8.tile([128, SQT], F32, tag="pr", name="pr", bufs=1)
                    nc.tensor.matmul(pr[:], r2t_sb[:], qsb)
                    t1 = sp.tile([128, SQT], BF16, tag="t1", name="t1")
                    nc.vector.tensor_mul(t1[:], qsb, cos_sb[:, ss])
                    t2 = sp.tile([128, SQT], BF16, tag="t2", name="t2")
                    nc.vector.tensor_mul(t2[:], pr[:], sin_sb[:, ss])
                    nc.vector.tensor_add(qrot[dt][:, ss], t1[:], t2[:])
                # rope on k (rows 0:64 of kv psum)
                ksb = sp.tile([64, SQT], BF16, tag="ksb", name="ksb")
                nc.vector.tensor_copy(ksb[:], pskv[0:64, :])
                prk_t = pool8.tile([128, SQT], F32, tag="pr", name="prk_t", bufs=1)
                prk = prk_t[0:64, :]
                nc.tensor.matmul(prk[:], r2t_sb[0:64, 0:64], ksb[:])
                t1k = sp.tile([64, SQT], BF16, tag="t1k", name="t1k")
                nc.vector.tensor_mul(t1k[:], ksb[:], cos_sb[0:64, ss])
                t2k = sp.tile([64, SQT], BF16, tag="t2k", name="t2k")
                nc.vector.tensor_mul(t2k[:], prk[:], sin_sb[0:64, ss])
                nc.vector.tensor_add(krot[0:64, ss], t1k[:], t2k[:])
                nc.vector.tensor_add(krot[64:128, ss], t1k[:], t2k[:])
                # V: transpose [64, 128] chunks -> vaug [128, 64] blocks
                vsb = sp.tile([64, SQT], BF16, tag="vsb", name="vsb")
                nc.vector.tensor_copy(vsb[:], pskv[64:128, :])
                for c in range(SQT // 128):
                    j = st * 4 + c  # global sk tile index
                    pt = pool8.tile([128, 64], BF16, tag="pr", name="pt", bufs=1)
                    nc.tensor.transpose(
                        pt[:], vsb[:, c * 128:(c + 1) * 128], id_sb[:]
                    )
                    nc.vector.tensor_copy(vaug[:, j * 65: j * 65 + 64], pt[:])

            # -- C phase: weight-stationary output projection ------------
            cq = deque()   # pending units: (b, sqt, ot)
            c_eng = [0]
            tail = [False]

            def emit_c_unit():
                b, sqt, ot = cq.popleft()
                t0 = b * S + sqt * SQT
                if tail[0]:
                    # A and B are done: the big tag's 2 ring slots join in
                    tag = ("pskv", "pr", "big", "big")[ot % 4]
                    bufs = 2 if tag == "big" else 1
                else:
                    tag = "pskv" if ot % 2 == 0 else "pr"
                    bufs = 1
                pw = pool8.tile([128, SQT], F32, tag=tag, name="pw", bufs=bufs)
                for jt in range(2):
                    nc.tensor.matmul(
                        pw[:],
                        wo_sb[jt][:, ot * 128:(ot + 1) * 128],
                        attnT[jt][:, t0:t0 + SQT],
                        start=(jt == 0),
                        stop=(jt == 1),
                    )
                osb = woutp.tile([128, SQT], BF16, tag="osb", name="osb")
                # scalar carries the serial exp chain during the attention
                # stream, so it only gets a third of the copies (half in
                # the drain tail where it idles)
                use_scalar = (c_eng[0] % 2 == 1) if tail[0] else (c_eng[0] % 3 == 2)
                if use_scalar:
                    nc.scalar.copy(osb[:], pw[:])
                else:
                    nc.vector.tensor_copy(osb[:], pw[:])
                c_eng[0] += 1
                # in the drain tail, split the final DMAs across the sync
                # and (idle) gpsimd queues so they don't serialize
                dma_eng = nc.gpsimd if (tail[0] and ot % 2 == 1) else nc.sync
                dma_eng.dma_start(
                    out[ot * 128:(ot + 1) * 128, t0:t0 + SQT], osb[:]
                )

            def push_c(b, sqt):
                for ot in range(DIM // 128):
                    cq.append((b, sqt, ot))

            def emit_b(b, sqt, reserve=8, tail_block=False):
                n_sk = 4 * (sqt + 1)
                total_steps = 2 * n_sk
                navail = len(cq)
                paced = max(0, navail - reserve)
                step = [0]
                emitted = [0]

                def weave():
                    step[0] += 1
                    want = paced * step[0] // total_steps
                    while emitted[0] < want:
                        emit_c_unit()
                        emitted[0] += 1

                for dt in range(2):
                    sq0 = b * S + sqt * SQT
                    po2 = pool8.tile([65, 2 * SQT], F32, tag="po2",
                                     name="po2", bufs=1)
                    for j in range(n_sk):
                        sk0 = b * S + j * SKT
                        d = j - 4 * sqt
                        off = max(0, 128 * d)  # causally dead columns
                        w = SQT - off
                        ps2 = pool8.tile([128, 2 * SQT], F32, tag="big",
                                         name="ps2", bufs=2)
                        for hp in range(2):
                            hs = slice(hp * 64, (hp + 1) * 64)
                            nc.tensor.matmul(
                                ps2[:, hp * SQT + off:(hp + 1) * SQT],
                                krot[hs, sk0:sk0 + SKT],
                                qrot[dt][hs, sq0 + off:sq0 + SQT],
                                tile_position=(hp * 64, 0),
                            )
                        et2 = ep.tile([128, 2 * SQT], BF16, tag="et2",
                                      name="et2")
                        if off == 0:
                            nc.scalar.activation(
                                et2[:], ps2[:],
                                mybir.ActivationFunctionType.Exp,
                                scale=0.125,
                            )
                        else:
                            pv = ps2[:].rearrange(
                                "p (h w) -> p h w", h=2)[:, :, off:SQT]
                            ev = et2[:].rearrange(
                                "p (h w) -> p h w", h=2)[:, :, off:SQT]
                            nc.scalar.activation(
                                ev, pv,
                                mybir.ActivationFunctionType.Exp,
                                scale=0.125,
                            )
                        if d >= 0:  # diagonal tile: zero sk > sq
                            for hp in range(2):
                                nc.gpsimd.affine_select(
                                    out=et2[:, hp * SQT + off:(hp + 1) * SQT],
                                    in_=et2[:, hp * SQT + off:(hp + 1) * SQT],
                                    compare_op=mybir.AluOpType.is_ge,
                                    fill=0.0,
                                    base=0,
                                    channel_multiplier=-1,
                                    pattern=[[1, w]],
                                )
                        jj = b * NSK + j
                        for hp in range(2):
                            nc.tensor.matmul(
                                po2[:, hp * SQT + off:(hp + 1) * SQT],
                                vaug[:, jj * 65:(jj + 1) * 65],
                                et2[:, hp * SQT + off:(hp + 1) * SQT],
                                start=(j == 0),
                                stop=(j == n_sk - 1),
                            )
                        weave()
                    # burst C units so the PE has independent work queued
                    # while po2 drains and the next dt block's first PV
                    # waits on the po2 bank
                    for _ in range(reserve // 2):
                        if cq:
                            emit_c_unit()
                            emitted[0] += 1
                    # drain po2 (pv rows AND denominator row) split across
                    # scalar and vector so the bank frees at the earlier of
                    # the two engine backlogs; the recip/broadcast/mul chain
                    # runs off SBUF afterwards
                    pocp = sp.tile([65, 2 * SQT], F32, tag="pocp", name="pocp",
                                   bufs=2)
                    nc.scalar.copy(pocp[:, 0:SQT], po2[:, 0:SQT])
                    nc.vector.tensor_copy(pocp[:, SQT:2 * SQT],
                                          po2[:, SQT:2 * SQT])
                    den2 = mp.tile([1, 2 * SQT], F32, tag="den", name="den",
                                   bufs=1)
                    nc.vector.tensor_copy(den2[:], pocp[64:65, :])
                    recip2 = mp.tile([1, 2 * SQT], F32, tag="recip",
                                     name="recip", bufs=1)
                    nc.vector.reciprocal_approx_fast(recip2[:], den2[:])
                    bc2 = mp.tile([64, 2 * SQT], F32, tag="bc", name="bc",
                                  bufs=1)
                    nc.gpsimd.partition_broadcast(bc2[:], recip2[:])
                    for hp in range(2):
                        nc.vector.tensor_mul(
                            attnT[dt][hp * 64:(hp + 1) * 64, sq0:sq0 + SQT],
                            pocp[0:64, hp * SQT:(hp + 1) * SQT],
                            bc2[:, hp * SQT:(hp + 1) * SQT],
                        )

            for st in range(4):
                emit_a(st)
            emit_b(0, 0)         # b=0 only needs st 0-3; starts the exp chain
            for st in range(4, 8):
                emit_a(st)
            push_c(0, 0); emit_b(0, 1)
            push_c(0, 1); emit_b(0, 2)
            push_c(0, 2); emit_b(0, 3)
            push_c(0, 3); emit_b(1, 0)
            push_c(1, 0); emit_b(1, 1)
            push_c(1, 1); emit_b(1, 2)
            push_c(1, 2); emit_b(1, 3, reserve=12, tail_block=True)
            push_c(1, 3)
            tail[0] = True
            while cq:
                emit_c_unit()

    nc.compile()
    nc.m = get_hw_module(nc.m)
    _CACHE["nc"] = nc
    return nc


def _prep_inputs(x, freqs_cos, freqs_sin, wq, wk, wv, wo):
    bf = ml_dtypes.bfloat16
    xT = np.ascontiguousarray(x.reshape(ST, DIM).T).astype(bf)
    # expanded rope tables in [feature, seq] layout, tiled over 2 head rows
    cos64 = np.repeat(freqs_cos.T, 2, axis=0)        # [64, S]
    sin64 = np.repeat(freqs_sin.T, 2, axis=0)
    cosE = np.tile(np.tile(cos64, (2, 1)), (1, B)).astype(bf)  # [128, ST]
    sinE = np.tile(np.tile(sin64, (2, 1)), (1, B)).astype(bf)
    # pair-swap matrix R (64x64), block-diagonal doubled, transposed
    R = np.zeros((64, 64), np.float32)
    for i in range(32):
        R[2 * i, 2 * i + 1] = -1.0
        R[2 * i + 1, 2 * i] = 1.0
    R2 = np.zeros((128, 128), np.float32)
    R2[:64, :64] = R
    R2[64:, 64:] = R
    r2t = np.ascontiguousarray(R2.T).astype(bf)
    ident = np.eye(64, dtype=np.float32).astype(bf)

    in_maps = []
    for c in range(N_CORES):
        wq_c = wq[c * QD:(c + 1) * QD, :]
        wk_c = wk[c * HD:(c + 1) * HD, :]
        wv_c = wv[c * HD:(c + 1) * HD, :]
        wkv_c = np.concatenate([wk_c, wv_c], axis=0)   # [128, DIM]
        wo_c = wo[:, c * QD:(c + 1) * QD]              # [DIM, 256]
        in_maps.append({
            "xt": xT,
            "wqt": np.ascontiguousarray(wq_c.T).astype(bf),
            "wkvt": np.ascontiguousarray(wkv_c.T).astype(bf),
            "wot": np.ascontiguousarray(wo_c.T).astype(bf),
            "cose": cosE,
            "sine": sinE,
            "r2t": r2t,
            "ident": ident,
        })
    return in_maps


def kernel(x, freqs_cos, freqs_sin, wq, wk, wv, wo, _trace=False, _trace_kwargs=None):
    x = np.asarray(x, dtype=np.float32)
    freqs_cos = np.asarray(freqs_cos, dtype=np.float32)
    freqs_sin = np.asarray(freqs_sin, dtype=np.float32)
    wq = np.asarray(wq, dtype=np.float32)
    wk = np.asarray(wk, dtype=np.float32)
    wv = np.asarray(wv, dtype=np.float32)
    wo = np.asarray(wo, dtype=np.float32)

    nc = _build()
    in_maps = _prep_inputs(x, freqs_cos, freqs_sin, wq, wk, wv, wo)
    kwargs = dict(_trace_kwargs or {})
    res = bass_utils.run_bass_kernel_spmd(
        nc, in_maps, core_ids=list(range(N_CORES)), trace=_trace, **kwargs
    )
    _CACHE["last_result"] = res
    acc = res.results[0]["out"].astype(np.float32)
    for c in range(1, N_CORES):
        acc += res.results[c]["out"].astype(np.float32)
    return np.ascontiguousarray(acc.T).reshape(B, S, DIM)
